# revision 44
# baseline (speedup 1.0000x reference)
"""Causal single-head attention (B=4, T=4096, D=1024, H=64) on 8 TRN2 cores, bf16.

Sharding: core c -> batch b=c//2, parity p=c%2; core owns the 16 interleaved
query tiles {128*(2i+p)}. All per-core differences live in input DATA (xT
column order, mask/threshold tiles, host-side output mapping) so the single
SPMD program is parity-free.

Device program: P1 = [Wk|Wq] @ x_own -> A1 (k-own rows 0:64 | q rows
64:128); P3 = [Wv|Wk] @ x_par -> A3. Span matmuls use full A1/A3 as
128x128 lhsT with zero-padded q operands (QZ/QH; junk weight rows hit
zero rhs rows) so FWL stays enabled -- without this the 2nd matmul of
each S^T/PV pair pays ~+100ns of serial LDWEIGHTS; v_sb is padded to
128 free (zeros beyond the ones column) for the same reason. Diagonal
straddle exp is EXACT (ACT exp + bf16 0/1 mask on DVE); off-diag slots
split ACT-exact / DVE Schraudolph per ACT_SLOTS. PV trails S^T by 3
slots (ppool bufs=4) to absorb exp-latency jitter, and each span's
PV-drain interleaves the next chunk's projections (drain pairs wait on
the last exps; accumulation groups are per-PSUM-bank so this is safe). x streams on the SP
HWDGE ring own-chunks-first (xc0,xc2,xc1,xc3,...) so P1/P2 of chunks
0/1 run while par data arrives; weights+tpos/thr ride the ACT ring
(small DMAs at the SP ring head stall the x stream). Startup zero
fills run on the idle GPSIMD engine; 10 warmup matmuls on a zeroed
tile flip the HAM clock gate to 2.4GHz before real data lands. The
TileContext teardown is patched to NOP sem-waits (serial Drains cost
~0.6us each) with no sem clearing (the NEFF executes once/process).
Output: pv PSUM -> o_sb in two halves, each DMAed immediately. Host
divides by the sumexp row (ones column) and transposes.
"""

import os
import re
import numpy as np
import ml_dtypes

B, T, D, H = 4, 4096, 1024, 64
NT = T // 128           # 32 key tiles per batch
NOWN = NT // 2          # 16 own query tiles per core
ND = D // 128           # 8 d-tiles
NSPAN = 4               # 4 spans of 512 own queries

SCHRAU_A = 0.125 * 128.0 / float(np.log(2.0))   # 23.0831...
BIAS_VIS = 16250.0
BIAS_MASK = 5600.0

_PROG = None
LAST_EXEC_TIME_NS = None
LAST_RESULTS = None


def _patch_tile_drain():
    """Walrus in this container allows only one sync-wait on NO_STRUCT
    instructions; TileContext's tail drain carries one wait per DMA lane.
    Split it into one drain per outstanding proc."""
    import bass_rust
    import concourse.tile as tile

    if getattr(tile.TileContext, "_drain_patched", False):
        return

    def _drain_and_barrier(self, tick_clock, wait_clock):
        nc = self.nc
        gvec = tick_clock.global_clock
        ticks = eval(re.match(r"VectorClock\((\[.*\])\)", repr(gvec)).group(1))
        # Cheap NOP sem-waits instead of serial Drain instructions (each
        # Drain costs ~0.6us; a satisfied NOP wait retires immediately).
        # The per-lane completion sems already imply all DMA data landed.
        for pr, tk in enumerate(ticks):
            if tk > 0:
                vec = [0] * len(ticks)
                vec[pr] = tk
                d = nc.sync.nop()
                wait_clock.add_sem_waits(
                    d.ins,
                    bass_rust.ScopedClock({None: bass_rust.VectorClock(vec)}),
                )
        nc.sync.drain()
        assert self.sems is not None
        popped = nc._tile_sem_poison_stack.pop()
        assert popped is self._sem_poison
        # No final all_engine_barrier and no clear_and_free_semaphores:
        # ordering is fully carried by data sems (o_sb -> outp DMA -> SP
        # NOP waits), the NEFF executes exactly once per process, and
        # ending the other engines' streams early lets the framework
        # epilogue overlap the final output DMA instead of serializing
        # behind it.

    tile.TileContext._drain_and_barrier = _drain_and_barrier
    tile.TileContext._drain_patched = True


def _split_multi_waits(nc):
    """This walrus build allows at most one sync-wait per instruction.
    Hoist extra waits onto injected same-engine NOPs placed just before the
    owning instruction (same engine stream => identical semantics)."""
    import bass_rust

    for bb in nc.main_func.blocks:
        new_list = []
        for ins in bb.instructions:
            si = ins.sync_info
            if si is not None and si.on_wait and len(si.on_wait) > 1:
                waits = list(si.on_wait)
                for w in waits[:-1]:
                    nop = nc.engines[ins.engine].nop().ins
                    for bb2 in nc.main_func.blocks:
                        if nop in bb2.instructions:
                            bb2.instructions.remove(nop)
                            break
                    nop.sync_info = bass_rust.SyncInfo(on_wait=[w], on_update=[])
                    new_list.append(nop)
                si.on_wait = [waits[-1]]
            new_list.append(ins)
        bb.instructions[:] = new_list


def _slot_order(j):
    """Interleave the 4 masked (diagonal) slots among the 4j unmasked ones.
    Masked slot leads its block so spans END on an unmasked slot (shorter
    exp chain before the final PV drain)."""
    un = list(range(4 * j))
    mk = [4 * j + u for u in range(4)]
    out = []
    per = len(un) // 4
    for u in range(4):
        out.append(mk[u])
        out.extend(un[u * per : (u + 1) * per])
    return out


# per span: set of unmasked slot indices whose exp runs on ACT (exact);
# the rest run DVE Schraudolph. Masked slots: own half always ACT-exact.
ACT_SLOTS = {0: set(), 1: {1, 3}, 2: {1, 4, 7}, 3: {1, 3, 6, 8, 11}}


def _build_program():
    import concourse.bass as bass
    import concourse.tile as tile
    from concourse import mybir
    _patch_tile_drain()
    f32 = mybir.dt.float32
    bf16 = mybir.dt.bfloat16
    i16 = mybir.dt.int16

    nc = bass.Bass()
    xC = [nc.dram_tensor(f"xc{i}", [128, ND, 512], bf16, kind="ExternalInput")
          for i in range(8)]  # own c0, par c0, own c1, par c1, ...
    wA = nc.dram_tensor("wA", [128, ND, 128], bf16, kind="ExternalInput")
    wV = nc.dram_tensor("wV", [128, ND, H], bf16, kind="ExternalInput")
    wP = nc.dram_tensor("wP", [128, ND, 128], bf16, kind="ExternalInput")
    tposd = nc.dram_tensor("tposd", [128, 512], i16, kind="ExternalInput")
    thrd = nc.dram_tensor("thrd", [128, 9], f32, kind="ExternalInput")
    outp = nc.dram_tensor("outp", [NSPAN, H + 1, 512], f32, kind="ExternalOutput")

    HLF = T // 2  # 2048
    au = mybir.AluOpType

    with tile.TileContext(nc) as tc:
        with (
            tc.tile_pool(name="singles", bufs=1) as singles,
            tc.tile_pool(name="pp", bufs=4) as ppool,
            tc.tile_pool(name="op", bufs=2) as opool,
            tc.tile_pool(name="psX", bufs=1, space="PSUM") as psX,
        ):
            # ---- SP ring: x only, in need-order (own0,par0,own1,...);
            # small head-of-ring DMAs would delay the x stream ~3us, so
            # weights/tpos/thr ride the ACT ring instead.
            xsb = [singles.tile([128, ND, 512], bf16, name=f"xs{i}")
                   for i in range(8)]
            # own chunks 0/1 lead (finely split), their par chunks follow:
            # P1/P2 of both chunks run while par data streams.
            for i in [0, 2, 1, 3, 4, 5, 6, 7]:
                step = 2 if i in (0, 2) else 4
                for d0 in range(0, ND, step):
                    nc.sync.dma_start(out=xsb[i][:, d0 : d0 + step, :],
                                      in_=xC[i][:, d0 : d0 + step, :])
            wA_sb = singles.tile([128, ND, 128], bf16)
            nc.scalar.dma_start(out=wA_sb, in_=wA[:, :, :])
            wV_sb = singles.tile([128, ND, H], bf16)
            nc.scalar.dma_start(out=wV_sb, in_=wV[:, :, :])
            wP_sb = singles.tile([128, ND, 128], bf16)
            nc.scalar.dma_start(out=wP_sb, in_=wP[:, :, :])
            tpos_sb = singles.tile([128, 512], i16)
            nc.scalar.dma_start(out=tpos_sb, in_=tposd[:, :])
            thr_sb = singles.tile([128, 9], f32)
            nc.scalar.dma_start(out=thr_sb, in_=thrd[:, :])

            # ---- PE warmup: ~3.4us of matmul activity flips the HAM clock
            # gate to 2.4GHz before the first data-dependent matmul arrives.
            wtile = singles.tile([128, 512], bf16)
            nc.vector.memset(wtile, 0.0)
            for w in range(8):
                pw = psX.tile([128, 512], f32, name="pw", tag="m1", bufs=3)
                nc.tensor.matmul(pw, lhsT=wtile[:, 0:128], rhs=wtile,
                                 start=True, stop=True)

            # ---- data-gated setup on DVE:
            # identity[s,t] = (tpos[s,t] == s) for t<128 (thr col 8 = s)
            ident = singles.tile([128, 128], bf16)
            nc.vector.tensor_scalar(
                out=ident, in0=tpos_sb[:, 0:128],
                scalar1=thr_sb[:, 8:9], scalar2=None, op0=au.is_equal)
            # own-diagonal 0/1 masks (bf16): visible iff tpos >= thr[u]
            mask_sb = singles.tile([128, 4, 512], bf16)
            for u in range(4):
                nc.vector.tensor_scalar(
                    out=mask_sb[:, u, :], in0=tpos_sb,
                    scalar1=thr_sb[:, u : u + 1], scalar2=None, op0=au.is_ge)
            # partner-half Schraudolph bias tiles (int16)
            bias_sb = singles.tile([128, 4, 512], i16)
            for u in range(4):
                nc.vector.tensor_scalar(
                    out=bias_sb[:, u, :], in0=tpos_sb,
                    scalar1=thr_sb[:, 4 + u : 5 + u], scalar2=None, op0=au.is_ge)
                nc.vector.tensor_scalar(
                    out=bias_sb[:, u, :], in0=bias_sb[:, u, :],
                    scalar1=BIAS_VIS - BIAS_MASK, scalar2=BIAS_MASK,
                    op0=au.mult, op1=au.add)

            A1 = singles.tile([128, HLF], bf16)    # k-own lo | q hi
            A3 = singles.tile([128, HLF], bf16)    # v-par lo | k-par hi
            # zero-padded q operands so span matmuls run full 128-contraction
            # 128-wide (FWL-eligible; avoids the ~+100ns serial-LDW penalty
            # on the 2nd matmul of each pair). Junk lhsT rows hit zero rhs
            # rows, contributing exactly 0.
            QZ = singles.tile([128, HLF], bf16)    # q lo | zeros hi
            QH = singles.tile([128, HLF], bf16)    # zeros lo | q hi
            # zero-fills on the otherwise-idle GPSIMD engine so the DVE
            # queue stays free for masks/dups (a DVE backlog here stalls
            # span 0's exp pipeline by several us)
            nc.gpsimd.memset(QZ[64:128, :], 0.0)
            nc.gpsimd.memset(QH[0:64, :], 0.0)
            VT = singles.tile([128, HLF], bf16)    # v-par lo | v-own hi
            v_sb = singles.tile([128, NOWN, 2, 128], bf16)
            nc.gpsimd.memset(v_sb, 0.0)
            nc.vector.memset(v_sb[:, :, :, H : H + 1], 1.0)

            def proj1(c):
                """[Wk|Wq] @ x_own and Wv @ x_own for chunk c."""
                xo = xsb[2 * c]
                sl = slice(0, 512)
                cs = slice(c * 512, (c + 1) * 512)
                ph = psX.tile([128, 512], f32, name="ph", tag="m1", bufs=3)
                for d in range(ND):
                    nc.tensor.matmul(ph, lhsT=wA_sb[:, d, :], rhs=xo[:, d, sl],
                                     start=(d == 0), stop=(d == ND - 1))
                nc.scalar.copy(out=A1[:, cs], in_=ph)
                # q dups for S^T rhs: QZ lo (partition shift -64 on DVE),
                # QH hi (same partitions, ACT)
                nc.vector.tensor_copy(out=QZ[0:64, cs], in_=A1[64:128, cs])
                nc.scalar.copy(out=QH[64:128, cs], in_=A1[64:128, cs])
                p2 = psX.tile([128, 512], f32, name="p2", tag="m1", bufs=3)
                for d in range(ND):
                    nc.tensor.matmul(p2[0:64, :], lhsT=wV_sb[:, d, :], rhs=xo[:, d, sl],
                                     start=(d == 0), stop=(d == ND - 1))
                nc.vector.tensor_copy(out=VT[64:128, cs], in_=p2[0:64, :])

            def proj3(c):
                """[Wv|Wk] @ x_par for chunk c."""
                xp = xsb[2 * c + 1]
                sl = slice(0, 512)
                cs = slice(c * 512, (c + 1) * 512)
                p3 = psX.tile([128, 512], f32, name="p3", tag="m1", bufs=3)
                for d in range(ND):
                    nc.tensor.matmul(p3, lhsT=wP_sb[:, d, :], rhs=xp[:, d, sl],
                                     start=(d == 0), stop=(d == ND - 1))
                # ACT, not DVE: keeps the deferred-chunk DVE queue short so
                # the preceding span's pv-releasing o_sb copy isn't delayed
                nc.scalar.copy(out=A3[:, cs], in_=p3)
                # v-par lo dup into VT (same partitions, scalar engine)
                nc.scalar.copy(out=VT[0:64, cs], in_=A3[0:64, cs])

            def transposes(g0, g1):
                for g in range(g0, g1):
                    tpt = psX.tile([128, 512], f32, name="tpt", tag="m1", bufs=3)
                    tpb = tpt[:, :].bitcast(bf16)[:, 0:128]
                    nc.tensor.transpose(tpb, VT[:, g * 128 : (g + 1) * 128], ident)
                    if g < 12:
                        nc.scalar.copy(out=v_sb[:, g, 0, 0:H], in_=tpb[:, 0:64])
                        nc.scalar.copy(out=v_sb[:, g, 1, 0:H], in_=tpb[:, 64:128])
                    else:
                        nc.vector.tensor_copy(out=v_sb[:, g, 0, 0:H], in_=tpb[:, 0:64])
                        nc.vector.tensor_copy(out=v_sb[:, g, 1, 0:H], in_=tpb[:, 64:128])

            def span(j, defer=None):
                qsl = slice(j * 512, (j + 1) * 512)
                order = _slot_order(j)
                act_set = ACT_SLOTS[j]
                n = len(order)
                pv = psX.tile([128, 512], f32, name="pv", tag="pv", bufs=1)
                pend = []
                for si, g in enumerate(order):
                    sc = psX.tile([128, 2, 512], f32, name="sc", tag="sc", bufs=2)
                    nc.tensor.matmul(sc[:, 0, :],
                                     lhsT=A1[:, g * 128 : (g + 1) * 128],
                                     rhs=QZ[:, qsl], start=True, stop=True)
                    nc.tensor.matmul(sc[:, 1, :],
                                     lhsT=A3[:, g * 128 : (g + 1) * 128],
                                     rhs=QH[:, qsl], start=True, stop=True)
                    p_t = ppool.tile([128, 2, 512], bf16, name="pt", tag="p")
                    if g >= 4 * j:  # diagonal straddle: exact exp + mask / bias
                        u = g - 4 * j
                        nc.scalar.activation(
                            out=p_t[:, 0, :], in_=sc[:, 0, :],
                            func=mybir.ActivationFunctionType.Exp, scale=0.125)
                        nc.vector.scalar_tensor_tensor(
                            out=p_t[:, 0, :], in0=p_t[:, 0, :],
                            scalar=1.0, in1=mask_sb[:, u, :],
                            op0=au.mult, op1=au.mult)
                        nc.vector.scalar_tensor_tensor(
                            out=p_t[:, 1, :].bitcast(i16), in0=sc[:, 1, :],
                            scalar=SCHRAU_A, in1=bias_sb[:, u, :],
                            op0=au.mult, op1=au.add)
                    elif g in act_set:
                        nc.scalar.activation(
                            out=p_t, in_=sc,
                            func=mybir.ActivationFunctionType.Exp, scale=0.125)
                    else:
                        nc.vector.tensor_scalar(
                            out=p_t.bitcast(i16), in0=sc,
                            scalar1=SCHRAU_A, scalar2=float(BIAS_VIS),
                            op0=au.mult, op1=au.add)
                    pend.append((g, p_t))
                    if len(pend) > 3:
                        g0, p0 = pend.pop(0)
                        nc.tensor.matmul(pv, lhsT=v_sb[:, g0, 1, :], rhs=p0[:, 0, :],
                                         start=(si == 3), stop=False)
                        nc.tensor.matmul(pv, lhsT=v_sb[:, g0, 0, :], rhs=p0[:, 1, :],
                                         start=False, stop=False)
                    del sc
                for k, (g0, p0) in enumerate(pend):
                    if k == 1 and defer is not None:
                        # interleave the next chunk's projections into the
                        # PV drain: the drain pairs wait on the last slots'
                        # exps, and these matmuls (other PSUM banks, data
                        # long since arrived) keep the PE busy meanwhile.
                        defer()
                    nc.tensor.matmul(pv, lhsT=v_sb[:, g0, 1, :], rhs=p0[:, 0, :],
                                     start=(n <= 2 and k == 0), stop=False)
                    nc.tensor.matmul(pv, lhsT=v_sb[:, g0, 0, :], rhs=p0[:, 1, :],
                                     start=False, stop=(k == len(pend) - 1))
                o_sb = opool.tile([H + 1, 512], f32, name="osb", tag="o")
                for hh in range(2):
                    hs = slice(hh * 256, (hh + 1) * 256)
                    nc.vector.tensor_copy(out=o_sb[:, hs], in_=pv[0 : H + 1, hs])
                    nc.sync.dma_start(out=outp[j][:, hs], in_=o_sb[:, hs])

            # chunks 0/1: both own-x projections first (their DMA leads),
            # par projections + spans fill the par-x streaming window;
            # later chunks' projections are deferred into the preceding
            # span's PV-drain window.
            proj1(0)
            proj1(1)
            proj3(0)
            transposes(0, 4)
            span(0, defer=lambda: (proj3(1), transposes(4, 8)))
            span(1, defer=lambda: (proj1(2), proj3(2), transposes(8, 12)))
            span(2, defer=lambda: (proj1(3), proj3(3), transposes(12, 16)))
            span(3)
    _split_multi_waits(nc)
    return nc


def _host_inputs(x, Wk, Wq, Wv):
    """Build the 8 per-core input maps (bf16/int16 device payloads)."""
    bf = ml_dtypes.bfloat16
    HLFH = T // 2
    maps = []
    def warr(w):  # [D, M] -> [128, ND, M] contiguous ([p][dt][m])
        return np.ascontiguousarray(
            w.reshape(ND, 128, -1).transpose(1, 0, 2)).astype(bf)
    wAh = warr(np.concatenate([Wk, Wq], axis=1))
    wPh = warr(np.concatenate([Wv, Wk], axis=1))
    wvh = warr(Wv)
    s = np.arange(128)
    t = np.arange(512)
    tpos = np.broadcast_to(((2 * (t // 128)) * 128 + (t % 128)).astype(np.int16),
                           (128, 512)).copy()
    for c in range(8):
        b, p = c // 2, c % 2
        own = [2 * i + p for i in range(NOWN)]
        oth = [2 * i + (1 - p) for i in range(NOWN)]
        own_rows = np.concatenate([np.arange(g * 128, (g + 1) * 128) for g in own])
        oth_rows = np.concatenate([np.arange(g * 128, (g + 1) * 128) for g in oth])
        xb = x[b]
        xTc = np.concatenate([xb[own_rows].T, xb[oth_rows].T], axis=1)  # [D, T]
        xTr = xTc.reshape(ND, 128, T).transpose(1, 0, 2)  # [128, ND, T]
        xcs = {}
        for cc in range(4):
            xcs[f"xc{2*cc}"] = np.ascontiguousarray(
                xTr[:, :, cc * 512 : (cc + 1) * 512]).astype(bf)
            xcs[f"xc{2*cc+1}"] = np.ascontiguousarray(
                xTr[:, :, HLFH + cc * 512 : HLFH + (cc + 1) * 512]).astype(bf)
        # thresholds: visible iff tpos >= thr
        # cols 0-3: own straddle u; cols 4-7: partner straddle u; col 8: s
        thr = np.zeros((128, 9), np.float32)
        for u in range(4):
            thr[:, u] = (2 * u) * 128 + s
            thr[:, 4 + u] = (2 * u + 1 - 2 * p) * 128 + s
        thr[:, 8] = s
        m = {"wA": wAh, "wV": wvh, "wP": wPh, "tposd": tpos, "thrd": thr}
        m.update(xcs)
        maps.append(m)
    return maps


def kernel(x, Wk, Wq, Wv):
    global _PROG, LAST_EXEC_TIME_NS, LAST_RESULTS
    from concourse.bass_utils import run_bass_kernel_spmd

    if _PROG is None:
        _PROG = _build_program()
    in_maps = _host_inputs(np.asarray(x, np.float32), np.asarray(Wk, np.float32),
                           np.asarray(Wq, np.float32), np.asarray(Wv, np.float32))
    trace = os.environ.get("BASS_KERNEL_TRACE", "0") == "1"
    res = run_bass_kernel_spmd(_PROG, in_maps, list(range(8)), trace=trace)
    LAST_EXEC_TIME_NS = res.exec_time_ns
    LAST_RESULTS = res
    out = np.zeros((B, T, H), np.float32)
    for c in range(8):
        b, p = c // 2, c % 2
        oc = res.results[c]["outp"].astype(np.float32)  # [4, 65, 512]
        for j in range(NSPAN):
            o = oc[j]
            on = (o[0:H, :] / o[H : H + 1, :]).T  # [512, 64]
            for u in range(4):
                gt = 8 * j + 2 * u + p
                out[b, gt * 128 : (gt + 1) * 128] = on[u * 128 : (u + 1) * 128]
    return out


# revision 45
# speedup vs baseline: 1.0063x; 1.0063x over previous
"""Causal single-head attention (B=4, T=4096, D=1024, H=64) on 8 TRN2 cores, bf16.

Sharding: core c -> batch b=c//2, parity p=c%2; core owns the 16 interleaved
query tiles {128*(2i+p)}. All per-core differences live in input DATA (xT
column order, mask/threshold tiles, host-side output mapping) so the single
SPMD program is parity-free.

Device program: P1 = [Wk|Wq] @ x_own -> A1 (k-own rows 0:64 | q rows
64:128); P3 = [Wv|Wk] @ x_par -> A3. Span matmuls use full A1/A3 as
128x128 lhsT with zero-padded q operands (QZ/QH; junk weight rows hit
zero rhs rows) so FWL stays enabled -- without this the 2nd matmul of
each S^T/PV pair pays ~+100ns of serial LDWEIGHTS; v_sb is padded to
128 free (zeros beyond the ones column) for the same reason. Diagonal
straddle exp is EXACT (ACT exp + bf16 0/1 mask on DVE); off-diag slots
split ACT-exact / DVE Schraudolph per ACT_SLOTS. PV trails S^T by 3
slots (ppool bufs=4) to absorb exp-latency jitter, and each span's
PV-drain interleaves the next chunk's projections (drain pairs wait on
the last exps; accumulation groups are per-PSUM-bank so this is safe). x streams on the SP
HWDGE ring own-chunks-first (xc0,xc2,xc1,xc3,...) so P1/P2 of chunks
0/1 run while par data arrives; weights+tpos/thr ride the ACT ring
(small DMAs at the SP ring head stall the x stream). Startup zero
fills run on the idle GPSIMD engine; 10 warmup matmuls on a zeroed
tile flip the HAM clock gate to 2.4GHz before real data lands. The
TileContext teardown is patched to NOP sem-waits (serial Drains cost
~0.6us each) with no sem clearing (the NEFF executes once/process).
Output: pv PSUM -> o_sb in two halves, each DMAed immediately. Host
divides by the sumexp row (ones column) and transposes.
"""

import os
import re
import numpy as np
import ml_dtypes

B, T, D, H = 4, 4096, 1024, 64
NT = T // 128           # 32 key tiles per batch
NOWN = NT // 2          # 16 own query tiles per core
ND = D // 128           # 8 d-tiles
NSPAN = 4               # 4 spans of 512 own queries

SCHRAU_A = 0.125 * 128.0 / float(np.log(2.0))   # 23.0831...
BIAS_VIS = 16250.0
BIAS_MASK = 5600.0

_PROG = None
LAST_EXEC_TIME_NS = None
LAST_RESULTS = None


def _patch_tile_drain():
    """Walrus in this container allows only one sync-wait on NO_STRUCT
    instructions; TileContext's tail drain carries one wait per DMA lane.
    Split it into one drain per outstanding proc."""
    import bass_rust
    import concourse.tile as tile

    if getattr(tile.TileContext, "_drain_patched", False):
        return

    def _drain_and_barrier(self, tick_clock, wait_clock):
        nc = self.nc
        gvec = tick_clock.global_clock
        ticks = eval(re.match(r"VectorClock\((\[.*\])\)", repr(gvec)).group(1))
        # Cheap NOP sem-waits instead of serial Drain instructions (each
        # Drain costs ~0.6us; a satisfied NOP wait retires immediately).
        # The per-lane completion sems already imply all DMA data landed.
        for pr, tk in enumerate(ticks):
            if tk > 0:
                vec = [0] * len(ticks)
                vec[pr] = tk
                d = nc.sync.nop()
                wait_clock.add_sem_waits(
                    d.ins,
                    bass_rust.ScopedClock({None: bass_rust.VectorClock(vec)}),
                )
        nc.sync.drain()
        assert self.sems is not None
        popped = nc._tile_sem_poison_stack.pop()
        assert popped is self._sem_poison
        # No final all_engine_barrier and no clear_and_free_semaphores:
        # ordering is fully carried by data sems (o_sb -> outp DMA -> SP
        # NOP waits), the NEFF executes exactly once per process, and
        # ending the other engines' streams early lets the framework
        # epilogue overlap the final output DMA instead of serializing
        # behind it.

    tile.TileContext._drain_and_barrier = _drain_and_barrier
    tile.TileContext._drain_patched = True


def _split_multi_waits(nc):
    """This walrus build allows at most one sync-wait per instruction.
    Hoist extra waits onto injected same-engine NOPs placed just before the
    owning instruction (same engine stream => identical semantics)."""
    import bass_rust

    for bb in nc.main_func.blocks:
        new_list = []
        for ins in bb.instructions:
            si = ins.sync_info
            if si is not None and si.on_wait and len(si.on_wait) > 1:
                waits = list(si.on_wait)
                for w in waits[:-1]:
                    nop = nc.engines[ins.engine].nop().ins
                    for bb2 in nc.main_func.blocks:
                        if nop in bb2.instructions:
                            bb2.instructions.remove(nop)
                            break
                    nop.sync_info = bass_rust.SyncInfo(on_wait=[w], on_update=[])
                    new_list.append(nop)
                si.on_wait = [waits[-1]]
            new_list.append(ins)
        bb.instructions[:] = new_list


def _slot_order(j):
    """Interleave the 4 masked (diagonal) slots among the 4j unmasked ones.
    Masked slot leads its block so spans END on an unmasked slot (shorter
    exp chain before the final PV drain)."""
    un = list(range(4 * j))
    mk = [4 * j + u for u in range(4)]
    out = []
    per = len(un) // 4
    for u in range(4):
        out.append(mk[u])
        out.extend(un[u * per : (u + 1) * per])
    return out


# per span: set of unmasked slot indices whose exp runs on ACT (exact);
# the rest run DVE Schraudolph. Masked slots: own half always ACT-exact.
ACT_SLOTS = {0: set(), 1: {1, 3}, 2: {1, 4, 7}, 3: {1, 3, 6, 8, 11}}


def _build_program():
    import concourse.bass as bass
    import concourse.tile as tile
    from concourse import mybir
    _patch_tile_drain()
    f32 = mybir.dt.float32
    bf16 = mybir.dt.bfloat16
    i16 = mybir.dt.int16

    nc = bass.Bass()
    xC = [nc.dram_tensor(f"xc{i}", [128, ND, 512], bf16, kind="ExternalInput")
          for i in range(8)]  # own c0, par c0, own c1, par c1, ...
    wA = nc.dram_tensor("wA", [128, ND, 128], bf16, kind="ExternalInput")
    wV = nc.dram_tensor("wV", [128, ND, H], bf16, kind="ExternalInput")
    wP = nc.dram_tensor("wP", [128, ND, 128], bf16, kind="ExternalInput")
    tposd = nc.dram_tensor("tposd", [128, 512], i16, kind="ExternalInput")
    thrd = nc.dram_tensor("thrd", [128, 9], f32, kind="ExternalInput")
    outp = nc.dram_tensor("outp", [NSPAN, H + 1, 512], f32, kind="ExternalOutput")

    HLF = T // 2  # 2048
    au = mybir.AluOpType

    with tile.TileContext(nc) as tc:
        with (
            tc.tile_pool(name="singles", bufs=1) as singles,
            tc.tile_pool(name="pp", bufs=4) as ppool,
            tc.tile_pool(name="op", bufs=2) as opool,
            tc.tile_pool(name="psX", bufs=1, space="PSUM") as psX,
        ):
            # ---- SP ring: x only, in need-order (own0,par0,own1,...);
            # small head-of-ring DMAs would delay the x stream ~3us, so
            # weights/tpos/thr ride the ACT ring instead.
            xsb = [singles.tile([128, ND, 512], bf16, name=f"xs{i}")
                   for i in range(8)]
            # own chunks 0/1 lead (finely split), their par chunks follow:
            # P1/P2 of both chunks run while par data streams.
            for i in [0, 2, 1, 3, 4, 5, 6, 7]:
                step = 2 if i in (0, 2) else 4
                for d0 in range(0, ND, step):
                    nc.sync.dma_start(out=xsb[i][:, d0 : d0 + step, :],
                                      in_=xC[i][:, d0 : d0 + step, :])
            wA_sb = singles.tile([128, ND, 128], bf16)
            nc.scalar.dma_start(out=wA_sb, in_=wA[:, :, :])
            wV_sb = singles.tile([128, ND, H], bf16)
            nc.scalar.dma_start(out=wV_sb, in_=wV[:, :, :])
            wP_sb = singles.tile([128, ND, 128], bf16)
            nc.scalar.dma_start(out=wP_sb, in_=wP[:, :, :])
            tpos_sb = singles.tile([128, 512], i16)
            nc.scalar.dma_start(out=tpos_sb, in_=tposd[:, :])
            thr_sb = singles.tile([128, 9], f32)
            nc.scalar.dma_start(out=thr_sb, in_=thrd[:, :])

            # ---- PE warmup: ~3.4us of matmul activity flips the HAM clock
            # gate to 2.4GHz before the first data-dependent matmul arrives.
            wtile = singles.tile([128, 512], bf16)
            nc.vector.memset(wtile, 0.0)
            for w in range(10):
                pw = psX.tile([128, 512], f32, name="pw", tag="m1", bufs=3)
                nc.tensor.matmul(pw, lhsT=wtile[:, 0:128], rhs=wtile,
                                 start=True, stop=True)

            # ---- data-gated setup on DVE:
            # identity[s,t] = (tpos[s,t] == s) for t<128 (thr col 8 = s)
            ident = singles.tile([128, 128], bf16)
            nc.vector.tensor_scalar(
                out=ident, in0=tpos_sb[:, 0:128],
                scalar1=thr_sb[:, 8:9], scalar2=None, op0=au.is_equal)
            # own-diagonal 0/1 masks (bf16): visible iff tpos >= thr[u]
            mask_sb = singles.tile([128, 4, 512], bf16)
            for u in range(4):
                nc.vector.tensor_scalar(
                    out=mask_sb[:, u, :], in0=tpos_sb,
                    scalar1=thr_sb[:, u : u + 1], scalar2=None, op0=au.is_ge)
            # partner-half Schraudolph bias tiles (int16)
            bias_sb = singles.tile([128, 4, 512], i16)
            for u in range(4):
                nc.vector.tensor_scalar(
                    out=bias_sb[:, u, :], in0=tpos_sb,
                    scalar1=thr_sb[:, 4 + u : 5 + u], scalar2=None, op0=au.is_ge)
                nc.vector.tensor_scalar(
                    out=bias_sb[:, u, :], in0=bias_sb[:, u, :],
                    scalar1=BIAS_VIS - BIAS_MASK, scalar2=BIAS_MASK,
                    op0=au.mult, op1=au.add)

            A1 = singles.tile([128, HLF], bf16)    # k-own lo | q hi
            A3 = singles.tile([128, HLF], bf16)    # v-par lo | k-par hi
            # zero-padded q operands so span matmuls run full 128-contraction
            # 128-wide (FWL-eligible; avoids the ~+100ns serial-LDW penalty
            # on the 2nd matmul of each pair). Junk lhsT rows hit zero rhs
            # rows, contributing exactly 0.
            QZ = singles.tile([128, HLF], bf16)    # q lo | zeros hi
            QH = singles.tile([128, HLF], bf16)    # zeros lo | q hi
            # zero-fills on the otherwise-idle GPSIMD engine so the DVE
            # queue stays free for masks/dups (a DVE backlog here stalls
            # span 0's exp pipeline by several us)
            nc.gpsimd.memset(QZ[64:128, :], 0.0)
            nc.gpsimd.memset(QH[0:64, :], 0.0)
            VT = singles.tile([128, HLF], bf16)    # v-par lo | v-own hi
            v_sb = singles.tile([128, NOWN, 2, 128], bf16)
            nc.gpsimd.memset(v_sb, 0.0)
            nc.vector.memset(v_sb[:, :, :, H : H + 1], 1.0)

            def proj1(c):
                """[Wk|Wq] @ x_own and Wv @ x_own for chunk c."""
                xo = xsb[2 * c]
                sl = slice(0, 512)
                cs = slice(c * 512, (c + 1) * 512)
                ph = psX.tile([128, 512], f32, name="ph", tag="m1", bufs=3)
                for d in range(ND):
                    nc.tensor.matmul(ph, lhsT=wA_sb[:, d, :], rhs=xo[:, d, sl],
                                     start=(d == 0), stop=(d == ND - 1))
                nc.scalar.copy(out=A1[:, cs], in_=ph)
                # q dups for S^T rhs: QZ lo (partition shift -64 on DVE),
                # QH hi (same partitions, ACT)
                nc.vector.tensor_copy(out=QZ[0:64, cs], in_=A1[64:128, cs])
                nc.scalar.copy(out=QH[64:128, cs], in_=A1[64:128, cs])
                p2 = psX.tile([128, 512], f32, name="p2", tag="m1", bufs=3)
                for d in range(ND):
                    nc.tensor.matmul(p2[0:64, :], lhsT=wV_sb[:, d, :], rhs=xo[:, d, sl],
                                     start=(d == 0), stop=(d == ND - 1))
                nc.vector.tensor_copy(out=VT[64:128, cs], in_=p2[0:64, :])

            def proj3(c):
                """[Wv|Wk] @ x_par for chunk c."""
                xp = xsb[2 * c + 1]
                sl = slice(0, 512)
                cs = slice(c * 512, (c + 1) * 512)
                p3 = psX.tile([128, 512], f32, name="p3", tag="m1", bufs=3)
                for d in range(ND):
                    nc.tensor.matmul(p3, lhsT=wP_sb[:, d, :], rhs=xp[:, d, sl],
                                     start=(d == 0), stop=(d == ND - 1))
                nc.vector.tensor_copy(out=A3[:, cs], in_=p3)
                # v-par lo dup into VT (same partitions, scalar engine)
                nc.scalar.copy(out=VT[0:64, cs], in_=A3[0:64, cs])

            def transposes(g0, g1):
                for g in range(g0, g1):
                    tpt = psX.tile([128, 512], f32, name="tpt", tag="m1", bufs=3)
                    tpb = tpt[:, :].bitcast(bf16)[:, 0:128]
                    nc.tensor.transpose(tpb, VT[:, g * 128 : (g + 1) * 128], ident)
                    if g < 12:
                        nc.scalar.copy(out=v_sb[:, g, 0, 0:H], in_=tpb[:, 0:64])
                        nc.scalar.copy(out=v_sb[:, g, 1, 0:H], in_=tpb[:, 64:128])
                    else:
                        nc.vector.tensor_copy(out=v_sb[:, g, 0, 0:H], in_=tpb[:, 0:64])
                        nc.vector.tensor_copy(out=v_sb[:, g, 1, 0:H], in_=tpb[:, 64:128])

            def span(j, defer=None):
                qsl = slice(j * 512, (j + 1) * 512)
                order = _slot_order(j)
                act_set = ACT_SLOTS[j]
                n = len(order)
                pv = psX.tile([128, 512], f32, name="pv", tag="pv", bufs=1)
                pend = []
                for si, g in enumerate(order):
                    sc = psX.tile([128, 2, 512], f32, name="sc", tag="sc", bufs=2)
                    nc.tensor.matmul(sc[:, 0, :],
                                     lhsT=A1[:, g * 128 : (g + 1) * 128],
                                     rhs=QZ[:, qsl], start=True, stop=True)
                    nc.tensor.matmul(sc[:, 1, :],
                                     lhsT=A3[:, g * 128 : (g + 1) * 128],
                                     rhs=QH[:, qsl], start=True, stop=True)
                    p_t = ppool.tile([128, 2, 512], bf16, name="pt", tag="p")
                    if g >= 4 * j:  # diagonal straddle: exact exp + mask / bias
                        u = g - 4 * j
                        nc.scalar.activation(
                            out=p_t[:, 0, :], in_=sc[:, 0, :],
                            func=mybir.ActivationFunctionType.Exp, scale=0.125)
                        nc.vector.scalar_tensor_tensor(
                            out=p_t[:, 0, :], in0=p_t[:, 0, :],
                            scalar=1.0, in1=mask_sb[:, u, :],
                            op0=au.mult, op1=au.mult)
                        nc.vector.scalar_tensor_tensor(
                            out=p_t[:, 1, :].bitcast(i16), in0=sc[:, 1, :],
                            scalar=SCHRAU_A, in1=bias_sb[:, u, :],
                            op0=au.mult, op1=au.add)
                    elif g in act_set:
                        nc.scalar.activation(
                            out=p_t, in_=sc,
                            func=mybir.ActivationFunctionType.Exp, scale=0.125)
                    else:
                        nc.vector.tensor_scalar(
                            out=p_t.bitcast(i16), in0=sc,
                            scalar1=SCHRAU_A, scalar2=float(BIAS_VIS),
                            op0=au.mult, op1=au.add)
                    pend.append((g, p_t))
                    if len(pend) > 3:
                        g0, p0 = pend.pop(0)
                        nc.tensor.matmul(pv, lhsT=v_sb[:, g0, 1, :], rhs=p0[:, 0, :],
                                         start=(si == 3), stop=False)
                        nc.tensor.matmul(pv, lhsT=v_sb[:, g0, 0, :], rhs=p0[:, 1, :],
                                         start=False, stop=False)
                    del sc
                for k, (g0, p0) in enumerate(pend):
                    if k == 1 and defer is not None:
                        # interleave the next chunk's projections into the
                        # PV drain: the drain pairs wait on the last slots'
                        # exps, and these matmuls (other PSUM banks, data
                        # long since arrived) keep the PE busy meanwhile.
                        defer()
                    nc.tensor.matmul(pv, lhsT=v_sb[:, g0, 1, :], rhs=p0[:, 0, :],
                                     start=(n <= 2 and k == 0), stop=False)
                    nc.tensor.matmul(pv, lhsT=v_sb[:, g0, 0, :], rhs=p0[:, 1, :],
                                     start=False, stop=(k == len(pend) - 1))
                o_sb = opool.tile([H + 1, 512], f32, name="osb", tag="o")
                for hh in range(2):
                    hs = slice(hh * 256, (hh + 1) * 256)
                    nc.vector.tensor_copy(out=o_sb[:, hs], in_=pv[0 : H + 1, hs])
                    nc.sync.dma_start(out=outp[j][:, hs], in_=o_sb[:, hs])

            # chunks 0/1: both own-x projections first (their DMA leads),
            # par projections + spans fill the par-x streaming window;
            # later chunks' projections are deferred into the preceding
            # span's PV-drain window.
            proj1(0)
            proj1(1)
            proj3(0)
            transposes(0, 4)
            span(0, defer=lambda: (proj3(1), transposes(4, 8)))
            span(1, defer=lambda: (proj1(2), proj3(2), transposes(8, 12)))
            span(2, defer=lambda: (proj1(3), proj3(3), transposes(12, 16)))
            span(3)
    _split_multi_waits(nc)
    return nc


def _host_inputs(x, Wk, Wq, Wv):
    """Build the 8 per-core input maps (bf16/int16 device payloads)."""
    bf = ml_dtypes.bfloat16
    HLFH = T // 2
    maps = []
    def warr(w):  # [D, M] -> [128, ND, M] contiguous ([p][dt][m])
        return np.ascontiguousarray(
            w.reshape(ND, 128, -1).transpose(1, 0, 2)).astype(bf)
    wAh = warr(np.concatenate([Wk, Wq], axis=1))
    wPh = warr(np.concatenate([Wv, Wk], axis=1))
    wvh = warr(Wv)
    s = np.arange(128)
    t = np.arange(512)
    tpos = np.broadcast_to(((2 * (t // 128)) * 128 + (t % 128)).astype(np.int16),
                           (128, 512)).copy()
    for c in range(8):
        b, p = c // 2, c % 2
        own = [2 * i + p for i in range(NOWN)]
        oth = [2 * i + (1 - p) for i in range(NOWN)]
        own_rows = np.concatenate([np.arange(g * 128, (g + 1) * 128) for g in own])
        oth_rows = np.concatenate([np.arange(g * 128, (g + 1) * 128) for g in oth])
        xb = x[b]
        xTc = np.concatenate([xb[own_rows].T, xb[oth_rows].T], axis=1)  # [D, T]
        xTr = xTc.reshape(ND, 128, T).transpose(1, 0, 2)  # [128, ND, T]
        xcs = {}
        for cc in range(4):
            xcs[f"xc{2*cc}"] = np.ascontiguousarray(
                xTr[:, :, cc * 512 : (cc + 1) * 512]).astype(bf)
            xcs[f"xc{2*cc+1}"] = np.ascontiguousarray(
                xTr[:, :, HLFH + cc * 512 : HLFH + (cc + 1) * 512]).astype(bf)
        # thresholds: visible iff tpos >= thr
        # cols 0-3: own straddle u; cols 4-7: partner straddle u; col 8: s
        thr = np.zeros((128, 9), np.float32)
        for u in range(4):
            thr[:, u] = (2 * u) * 128 + s
            thr[:, 4 + u] = (2 * u + 1 - 2 * p) * 128 + s
        thr[:, 8] = s
        m = {"wA": wAh, "wV": wvh, "wP": wPh, "tposd": tpos, "thrd": thr}
        m.update(xcs)
        maps.append(m)
    return maps


def kernel(x, Wk, Wq, Wv):
    global _PROG, LAST_EXEC_TIME_NS, LAST_RESULTS
    from concourse.bass_utils import run_bass_kernel_spmd

    if _PROG is None:
        _PROG = _build_program()
    in_maps = _host_inputs(np.asarray(x, np.float32), np.asarray(Wk, np.float32),
                           np.asarray(Wq, np.float32), np.asarray(Wv, np.float32))
    trace = os.environ.get("BASS_KERNEL_TRACE", "0") == "1"
    res = run_bass_kernel_spmd(_PROG, in_maps, list(range(8)), trace=trace)
    LAST_EXEC_TIME_NS = res.exec_time_ns
    LAST_RESULTS = res
    out = np.zeros((B, T, H), np.float32)
    for c in range(8):
        b, p = c // 2, c % 2
        oc = res.results[c]["outp"].astype(np.float32)  # [4, 65, 512]
        for j in range(NSPAN):
            o = oc[j]
            on = (o[0:H, :] / o[H : H + 1, :]).T  # [512, 64]
            for u in range(4):
                gt = 8 * j + 2 * u + p
                out[b, gt * 128 : (gt + 1) * 128] = on[u * 128 : (u + 1) * 128]
    return out
